# revision 39
# baseline (speedup 1.0000x reference)
"""EnhancedGCN on 8 Trainium2 NeuronCores (Bass/Tile, SPMD).

Strategy: 1D node partition (6250 nodes/core, padded to 6272). Small weights
replicated. Per propagation step: each core computes xws = dis * (h @ conv_w.T)
for its nodes, AllGathers the bf16 table (4 collectives writing slices of two
int16-addressable super-tables, pipelined behind the xws windows), then
gathers source rows per edge (dma_gather over 4 SWDGE queues), reduces them
into per-target sums with 0/1 selection-matrix matmuls accumulating in PSUM
(self-loops enter as an identity-matmul block), and applies the pointwise
epilogue (degree norm, root/relu term, residual+LN between steps).
Edge weights ew = dis[t]*dis[s] are separable: they fold into a pre-scale of
the table (dis[s]) and a post-scale of the message sum (dis[t]).
Host-side work is limited to graph-structure prep (sorting edges into
target windows, block padding, int16 index streams) and weight transposes.
"""
import sys

sys.path.insert(0, "/opt/trn_rl_repo")

import numpy as np
import ml_dtypes

import concourse.bass as bass
import concourse.bacc as bacc
import concourse.tile as tile
import concourse.mybir as mybir
from concourse.bass_utils import run_bass_kernel_spmd
from concourse.masks import make_identity

BF16 = ml_dtypes.bfloat16
N, IN, H = 50000, 256, 128
NCORES = 8
NPC = N // NCORES  # 6250
NW = (NPC + 127) // 128  # 49
PADN = NW * 128  # 6272
LN_EPS = 1e-5
NGRP = (NW + 3) // 4  # 13 groups of 4 windows

# Two gather super-streams, one AllGather each (Shared output, single writer).
NSUP = 2
SUP_W = [24, 25]           # windows per super
SUP_W0 = [0, 24, 49]       # window ranges: super 0 = [0,24), super 1 = [24,49)
SUP_SZ = [w * 128 for w in SUP_W]  # per-rank rows per super
# table row layout per super: rank-major [8 ranks x SUP_SZ]
TBL_ROWS = [8 * SUP_SZ[0], 8 * SUP_SZ[1]]

F32 = mybir.dt.float32
BF = mybir.dt.bfloat16
I16 = mybir.dt.int16
AX = mybir.AluOpType
AF = mybir.ActivationFunctionType


def _bcast_mid(ap, n):
    """[128, F] AP -> [128, n, F] with stride-0 middle dim."""
    a = ap.copy()
    a.ap = [a.ap[0], [0, n]] + a.ap[1:]
    return a


def _bcast_last(ap, n):
    """[128, G] AP -> [128, G, n] with stride-0 last dim."""
    a = ap.copy()
    a.ap = a.ap + [[0, n]]
    return a


def _r3(ap, f):
    return ap.rearrange("p (w f) -> p w f", f=f)


def _wrap_idx(idx):
    """flat idx [n] (n % 16 == 0) -> [128, n/16] int16 wrapped + replicated."""
    n = len(idx)
    t = idx.reshape(n // 16, 16).T.astype(np.int16)
    return np.tile(t, (8, 1))


def _prep_graph(row, col):
    """Graph-structure-only preprocessing (row/col ints)."""
    deg = np.bincount(row, minlength=N).astype(np.float64) + 1.0
    dis_f = 1.0 / np.sqrt(deg)
    dinv_f = 1.0 / deg

    core = row // NPC
    src_core = col // NPC
    src_off = col % NPC
    src_w = src_off >> 7
    src_sup = (src_w >= SUP_W[0]).astype(np.int64)
    ssz = np.asarray(SUP_SZ)[src_sup]
    w0 = np.asarray(SUP_W0)[src_sup] * 128
    src_idx = src_core * ssz + (src_off - w0)

    per_core = []
    counts = np.zeros((NCORES, NW, NSUP), np.int64)
    for k in range(NCORES):
        m = core == k
        tgt = (row[m] - k * NPC).astype(np.int64)
        sidx = src_idx[m]
        ssup = src_sup[m]
        w = tgt >> 7
        order = np.argsort(w, kind="stable")
        tgt, sidx, ssup, w = tgt[order], sidx[order], ssup[order], w[order]
        ents = []
        bounds = np.searchsorted(w, np.arange(NW + 1))
        for wi in range(NW):
            sl = slice(bounds[wi], bounds[wi + 1])
            s_w, t_w, u_w = sidx[sl], tgt[sl] - (wi << 7), ssup[sl]
            by_sup = []
            for s in range(NSUP):
                mm = u_w == s
                by_sup.append((s_w[mm], t_w[mm]))
                counts[k, wi, s] = int(mm.sum())
            ents.append(by_sup)
        per_core.append(ents)

    B = np.maximum(np.ceil(counts.max(axis=0) / 128), 1).astype(np.int64)  # [NW, NSUP]
    NBLK = B.sum(axis=0).astype(np.int64)  # per super
    gpos = np.zeros((NSUP, NGRP + 1), np.int64)
    for s in range(NSUP):
        pref = np.concatenate([[0], np.cumsum(B[:, s])])
        for g in range(NGRP + 1):
            gpos[s, g] = pref[min(g * 4, NW)]

    # pad slots read random table rows (spread across HBM banks; a fixed pad
    # row measurably serializes the gather queues on one bank)
    rng = np.random.default_rng(12345)
    idx_streams = [np.empty((NCORES, int(NBLK[s]) * 128), np.int64) for s in range(NSUP)]
    for s in range(NSUP):
        idx_streams[s][:] = rng.integers(0, TBL_ROWS[s], idx_streams[s].shape)
    tlocs = [np.full((NCORES, 128, int(NBLK[s])), -1.0, np.float32) for s in range(NSUP)]

    for k in range(NCORES):
        pos = [0] * NSUP
        for wi in range(NW):
            for s in range(NSUP):
                s_w, t_w = per_core[k][wi][s]
                n = len(s_w)
                p = pos[s]
                idx_streams[s][k, p * 128 : p * 128 + n] = s_w
                j = np.arange(n)
                tlocs[s][k, j % 128, p + j // 128] = t_w
                pos[s] += int(B[wi, s])



    # wrap idx per half-call segment (aligned to group boundaries, split in two)
    idx_w = [None] * NSUP
    for s in range(NSUP):
        per_core_w = [[] for _ in range(NCORES)]
        for g in range(NGRP):
            b0, b1 = int(gpos[s, g]), int(gpos[s, g + 1])
            mid = b0 + (b1 - b0 + 1) // 2
            for (h0_, h1_) in ((b0, mid), (mid, b1)):
                if h1_ > h0_:
                    for k in range(NCORES):
                        per_core_w[k].append(
                            _wrap_idx(idx_streams[s][k, h0_ * 128 : h1_ * 128])
                        )
        idx_w[s] = np.stack([np.concatenate(x, axis=1) for x in per_core_w])

    dis_cols = np.zeros((NCORES, 128, NW), np.float32)
    dinv_cols = np.ones((NCORES, 128, NW), np.float32)
    for k in range(NCORES):
        v = np.zeros(PADN, np.float64)
        v[:NPC] = dis_f[k * NPC : (k + 1) * NPC]
        dis_cols[k] = v.reshape(NW, 128).T
        u = np.ones(PADN, np.float64)
        u[:NPC] = dinv_f[k * NPC : (k + 1) * NPC]
        dinv_cols[k] = u.reshape(NW, 128).T

    return dict(
        B=B,
        NBLK=NBLK,
        gpos=gpos,
        idx_w=idx_w,
        tlocs=[t.astype(BF16) for t in tlocs],
        dis_cols=dis_cols,
        dinv_cols=dinv_cols,
    )


def _build(B, gpos, NBLK):
    nc = bacc.Bacc("TRN2", target_bir_lowering=False, debug=False, num_swdge_queues=4)

    ift = nc.dram_tensor("ift", [IN, PADN], BF, kind="ExternalInput")
    lin_wT = nc.dram_tensor("lin_wT", [IN, H], BF, kind="ExternalInput")
    conv_wT = nc.dram_tensor("conv_wT", [H, H], BF, kind="ExternalInput")
    consts = nc.dram_tensor("consts", [128, 5 * H], F32, kind="ExternalInput")
    iota_in = nc.dram_tensor("iota", [128, 128], BF, kind="ExternalInput")
    discols = nc.dram_tensor("discols", [128, NW], F32, kind="ExternalInput")
    dinvcols = nc.dram_tensor("dinvcols", [128, NW], F32, kind="ExternalInput")
    idx_t = [
        nc.dram_tensor(f"idx{s}", [128, int(NBLK[s]) * 8], I16, kind="ExternalInput")
        for s in range(NSUP)
    ]
    tloc_t = [
        nc.dram_tensor(f"tloc{s}", [128, int(NBLK[s])], BF, kind="ExternalInput")
        for s in range(NSUP)
    ]
    out_ext = nc.dram_tensor("out", [PADN, H], F32, kind="ExternalOutput")

    def ws(w):
        return slice(w * 128, (w + 1) * 128)

    # per-(stream, group) half-call boundaries + column offset into wrapped idx
    halves = {}
    for s in range(NSUP):
        off = 0
        for g in range(NGRP):
            b0, b1 = int(gpos[s, g]), int(gpos[s, g + 1])
            mid = b0 + (b1 - b0 + 1) // 2
            hs = []
            for (h0_, h1_) in ((b0, mid), (mid, b1)):
                hs.append((h0_, h1_, off))
                off += (h1_ - h0_) * 8
            halves[(s, g)] = hs
    gmax = max(h1 - h0 for v in halves.values() for (h0, h1, _) in v)
    wgmax = int(
        max(sum(int(gpos[s, g + 1] - gpos[s, g]) for s in range(NSUP)) for g in range(NGRP))
    )

    with tile.TileContext(nc) as tc:
        with (
            tc.tile_pool(name="const", bufs=1) as cpool,
            tc.tile_pool(name="state", bufs=1) as spool,
            tc.tile_pool(name="iftp", bufs=1) as ipool,
            tc.tile_pool(name="ht", bufs=3) as hpool,
            tc.tile_pool(name="gath", bufs=12) as gpool,
            tc.tile_pool(name="wp", bufs=2) as wpool,
            tc.tile_pool(name="tmp", bufs=1) as tpool,
            tc.tile_pool(name="psP", bufs=3, space="PSUM") as psP,
            tc.tile_pool(name="psM", bufs=4, space="PSUM") as psM,
            tc.tile_pool(name="dram", bufs=1, space="DRAM") as dpool,
        ):
            identf = cpool.tile([128, 128], F32)
            make_identity(nc, identf[:])
            identb = cpool.tile([128, 128], BF)
            nc.vector.tensor_copy(out=identb[:], in_=identf[:])
            cst = cpool.tile([128, 5 * H], F32)
            nc.sync.dma_start(out=cst[:], in_=consts[:])
            linb, rootr, convbr, g1r, b1r = (cst[:, i * H : (i + 1) * H] for i in range(5))
            iot = cpool.tile([128, 128], BF)
            nc.sync.dma_start(out=iot[:], in_=iota_in[:])
            cw = cpool.tile([128, H], BF)
            nc.sync.dma_start(out=cw[:], in_=conv_wT[:])
            lw0 = cpool.tile([128, H], BF)
            nc.sync.dma_start(out=lw0[:], in_=lin_wT[0:128, :])
            lw1 = cpool.tile([128, H], BF)
            nc.sync.dma_start(out=lw1[:], in_=lin_wT[128:256, :])
            dic = cpool.tile([128, NW], F32)
            nc.sync.dma_start(out=dic[:], in_=discols[:])
            dvc = cpool.tile([128, NW], F32)
            nc.sync.dma_start(out=dvc[:], in_=dinvcols[:])
            tl_sb = []
            idx_sb = []
            for s in range(NSUP):
                t = cpool.tile([128, int(NBLK[s])], BF, name=f"tl{s}")
                nc.sync.dma_start(out=t[:], in_=tloc_t[s][:])
                tl_sb.append(t)
                t2 = cpool.tile([128, int(NBLK[s]) * 8], I16, name=f"ix{s}")
                nc.sync.dma_start(out=t2[:], in_=idx_t[s][:])
                idx_sb.append(t2)

            h0 = spool.tile([128, PADN], F32, tag="h0")
            hA = spool.tile([128, PADN], F32, tag="hA")
            hB = spool.tile([128, PADN], F32, tag="hB")
            xws0 = spool.tile([128, PADN], BF, tag="xws0")
            xws1 = spool.tile([128, PADN], BF, tag="xws1")

            ctxs = {}

            def ensure_ctx(s_step):
                if s_step not in ctxs:
                    tbA = dpool.tile(
                        [TBL_ROWS[0], H], BF, tag=f"tb{s_step}_0", name=f"tbA{s_step}",
                        addr_space="Shared",
                    )
                    tbB = dpool.tile(
                        [TBL_ROWS[1], H], BF, tag=f"tb{s_step}_1", name=f"tbB{s_step}",
                        addr_space="Shared",
                    )
                    ctxs[s_step] = dict(tb=[tbA, tbB], call_tiles={}, w_tiles={}, blkpos=[0] * NSUP)
                return ctxs[s_step]

            def pub_ln_g(g):
                """step-1 LN+relu for group g's windows (hA+h0 -> hB)."""
                if True:
                    if True:
                        gw = min(4, NW - 4 * g)
                        sl = slice(4 * g * 128, (4 * g + gw) * 128)
                        X_t = tpool.tile([128, 4 * 128], F32, tag="ln_X")
                        X = X_t[:, : gw * 128]
                        Y_t = tpool.tile([128, 4 * 128], F32, tag="ln_Y")
                        Y = Y_t[:, : gw * 128]
                        nc.vector.tensor_tensor(out=X, in0=hA[:, sl], in1=h0[:, sl], op=AX.add)
                        mu_t = tpool.tile([128, 4], F32, tag="ln_mu")
                        mu = mu_t[:, :gw]
                        nc.vector.tensor_reduce(out=mu, in_=_r3(X, 128), axis=mybir.AxisListType.X, op=AX.add)
                        nc.vector.tensor_scalar_mul(out=mu, in0=mu, scalar1=1.0 / 128.0)
                        nc.vector.tensor_tensor(out=Y, in0=X, in1=X, op=AX.mult)
                        var_t = tpool.tile([128, 4], F32, tag="ln_var")
                        var = var_t[:, :gw]
                        nc.vector.tensor_reduce(out=var, in_=_r3(Y, 128), axis=mybir.AxisListType.X, op=AX.add)
                        mm_t = tpool.tile([128, 4], F32, tag="ln_mm")
                        mm = mm_t[:, :gw]
                        nc.vector.tensor_tensor(out=mm, in0=mu, in1=mu, op=AX.mult)
                        nc.vector.tensor_scalar(
                            out=var, in0=var, scalar1=1.0 / 128.0, scalar2=LN_EPS, op0=AX.mult, op1=AX.add
                        )
                        nc.vector.tensor_tensor(out=var, in0=var, in1=mm, op=AX.subtract)
                        sd_t = tpool.tile([128, 4], F32, tag="ln_sd")
                        sd = sd_t[:, :gw]
                        nc.scalar.activation(out=sd, in_=var, func=AF.Sqrt)
                        rstd_t = tpool.tile([128, 4], F32, tag="ln_rs")
                        rstd = rstd_t[:, :gw]
                        nc.vector.reciprocal(out=rstd, in_=sd)
                        mb_t = tpool.tile([128, 4], F32, tag="ln_mb")
                        mb = mb_t[:, :gw]
                        nc.vector.tensor_tensor(out=mb, in0=mu, in1=rstd, op=AX.mult)
                        nc.vector.tensor_scalar_mul(out=mb, in0=mb, scalar1=-1.0)
                        nc.vector.tensor_tensor(
                            out=_r3(Y, 128), in0=_r3(X, 128),
                            in1=_bcast_last(rstd, 128), op=AX.mult,
                        )
                        nc.vector.tensor_tensor(
                            out=_r3(X, 128), in0=_r3(Y, 128),
                            in1=_bcast_last(mb, 128), op=AX.add,
                        )
                        nc.vector.tensor_tensor(out=_r3(Y, 128), in0=_r3(X, 128), in1=_bcast_mid(g1r, gw), op=AX.mult)
                        nc.vector.tensor_tensor(out=_r3(X, 128), in0=_r3(Y, 128), in1=_bcast_mid(b1r, gw), op=AX.add)
                        nc.scalar.activation(out=hB[:, sl], in_=X, func=AF.Relu)

            def pub_xws_g(s_step, g):
                """transpose + conv + dis-scale for group g's windows -> xws."""
                st = hB if s_step == 1 else h0
                xws = xws1 if s_step == 1 else xws0
                g0 = 4 * g
                gw = min(4, NW - g0)
                tpg = psP.tile([128, 4 * 128], F32, tag="pg")
                for wq in range(gw):
                    nc.tensor.transpose(
                        tpg[:, wq * 128 : (wq + 1) * 128], st[:, ws(g0 + wq)], identf[:]
                    )
                htg = hpool.tile([128, 4 * 128], BF, tag="ht")
                nc.scalar.copy(out=htg[:, : gw * 128], in_=tpg[:, : gw * 128])
                xpg = psP.tile([128, 4 * 128], F32, tag="pg")
                for wq in range(gw):
                    nc.tensor.matmul(
                        xpg[:, wq * 128 : (wq + 1) * 128],
                        lhsT=htg[:, wq * 128 : (wq + 1) * 128],
                        rhs=cw[:], start=True, stop=True,
                    )
                nc.vector.tensor_tensor(
                    out=_r3(xws[:, g0 * 128 : (g0 + gw) * 128], 128),
                    in0=_r3(xpg[:, : gw * 128], 128),
                    in1=_bcast_last(dic[:, g0 : g0 + gw], 128),
                    op=AX.mult,
                )

            def pub_ag(s_step, sup):
                ctx = ensure_ctx(s_step)
                xws = xws1 if s_step == 1 else xws0
                w0c, w1c = SUP_W0[sup], SUP_W0[sup + 1]
                szS = SUP_SZ[sup]
                lx = dpool.tile([szS, H], BF, tag=f"lx{s_step}_{sup}", name=f"lx{s_step}_{sup}")
                nc.sync.dma_start(
                    out=lx[:].rearrange("(w p) f -> p w f", p=128),
                    in_=_r3(xws[:, w0c * 128 : w1c * 128], 128),
                )
                nc.gpsimd.collective_compute(
                    "AllGather",
                    AX.bypass,
                    replica_groups=[list(range(NCORES))],
                    ins=[lx.opt()],
                    outs=[ctx["tb"][sup][:]],
                )

            def emit_publish0(sup):
                """step-0: h-linear + xws + publish + AllGather for super sup."""
                ensure_ctx(0)
                w0c, w1c = SUP_W0[sup], SUP_W0[sup + 1]
                for sw0 in range(w0c, w1c, 12):
                    sw1 = min(sw0 + 12, w1c)
                    scw = (sw1 - sw0) * 128
                    ichunk = ipool.tile([128, 2 * 12 * 128], BF, tag="ift")
                    nc.sync.dma_start(
                        out=ichunk[:, :scw], in_=ift[0:128, sw0 * 128 : sw1 * 128]
                    )
                    nc.sync.dma_start(
                        out=ichunk[:, scw : 2 * scw],
                        in_=ift[128:256, sw0 * 128 : sw1 * 128],
                    )
                    for g in range(sw0, sw1, 4):
                        gw2 = min(4, sw1 - g)
                        hpg = psP.tile([128, 4 * 128], F32, tag="pg")
                        for wq in range(gw2):
                            woff = (g + wq - sw0) * 128
                            dsl = slice(wq * 128, (wq + 1) * 128)
                            nc.tensor.matmul(
                                hpg[:, dsl], lhsT=ichunk[:, woff : woff + 128],
                                rhs=lw0[:], start=True, stop=False,
                            )
                            nc.tensor.matmul(
                                hpg[:, dsl],
                                lhsT=ichunk[:, scw + woff : scw + woff + 128],
                                rhs=lw1[:], start=False, stop=True,
                            )
                        nc.vector.tensor_tensor(
                            out=_r3(h0[:, g * 128 : (g + gw2) * 128], 128),
                            in0=_r3(hpg[:, : gw2 * 128], 128),
                            in1=_bcast_mid(linb, gw2), op=AX.add,
                        )
                for g in range(w0c // 4, (w1c + 3) // 4):
                    pub_xws_g(0, g)
                pub_ag(0, sup)

            def call_tile(s_step, s, g, h):
                ctx = ctxs[s_step]
                key = (s, g, h)
                if key not in ctx["call_tiles"]:
                    h0_, h1_, off = halves[(s, g)][h]
                    nb = h1_ - h0_
                    if nb == 0:
                        ctx["call_tiles"][key] = None
                    else:
                        gt = gpool.tile([128, gmax * H], BF, tag="gath")
                        nc.gpsimd.dma_gather(
                            gt[:, : nb * H].rearrange("p (b e) -> p b e", e=H),
                            ctx["tb"][s][:],
                            idx_sb[s][:, off : off + nb * 8],
                            nb * 128,
                            nb * 128,
                            H,
                            single_packet=False,
                            queue_num=(2 * s + 2 * g + h) % 4,
                        )
                        ctx["call_tiles"][key] = gt
                return ctx["call_tiles"][key]

            def w_tile(s_step, g):
                ctx = ctxs[s_step]
                if g not in ctx["w_tiles"]:
                    wt = wpool.tile([128, wgmax * 128], BF, tag="W")
                    offs = []
                    o = 0
                    for s in range(NSUP):
                        nb = int(gpos[s, g + 1] - gpos[s, g])
                        offs.append(o)
                        if nb > 0:
                            nc.vector.tensor_tensor(
                                out=_r3(wt[:, o * 128 : (o + nb) * 128], 128),
                                in0=tl_sb[s][:, int(gpos[s, g]) : int(gpos[s, g + 1])].to_broadcast([128, nb, 128]),
                                in1=_bcast_mid(iot[:], nb),
                                op=AX.is_equal,
                            )
                        o += nb
                    ctx["w_tiles"][g] = (wt, offs)
                return ctx["w_tiles"][g]

            def emit_groups(s_step, glo, ghi):
                ctx = ensure_ctx(s_step)
                state = hB if s_step == 1 else h0
                xws_s = xws1 if s_step == 1 else xws0
                hdst = hA
                pub1 = s_step == 0
                for grp in range(glo, ghi):
                    bg = grp * 4
                    for gg in (grp, grp + 1, grp + 2):
                        if gg < NGRP:
                            for s in range(NSUP):
                                call_tile(s_step, s, gg, 0)
                                call_tile(s_step, s, gg, 1)
                    for gg in (grp, grp + 1):
                        if gg < NGRP:
                            w_tile(s_step, gg)
                    gw = min(4, NW - bg)
                    pm = psM.tile([128, 4 * 128], F32, tag="msg")
                    nc.tensor.matmul(
                        pm[:, : gw * 128], lhsT=identb[:],
                        rhs=xws_s[:, bg * 128 : (bg + gw) * 128],
                        start=True, stop=False,
                    )
                    for wq in range(gw):
                        w = bg + wq
                        dst = pm[:, wq * 128 : (wq + 1) * 128]
                        nblk = int(B[w].sum())
                        bi = 0
                        for s in range(NSUP):
                            for _ in range(int(B[w, s])):
                                gidx = ctx["blkpos"][s]
                                hh = halves[(s, grp)]
                                h = 0 if gidx < hh[0][1] else 1
                                h0_, h1_, _off = hh[h]
                                ct = call_tile(s_step, s, grp, h)
                                loc = gidx - h0_
                                wt_, woffs = w_tile(s_step, grp)
                                wloc = woffs[s] + (gidx - int(gpos[s, grp]))
                                nc.tensor.matmul(
                                    dst,
                                    lhsT=wt_[:, wloc * 128 : (wloc + 1) * 128],
                                    rhs=ct[:].rearrange("p (b e) -> p b e", e=H)[:, loc, :],
                                    start=False,
                                    stop=(bi == nblk - 1),
                                )
                                ctx["blkpos"][s] += 1
                                bi += 1
                    sl = slice(bg * 128, (bg + gw) * 128)
                    E1_t = tpool.tile([128, 4 * 128], F32, tag="ep_E1")
                    E1 = E1_t[:, : gw * 128]
                    E2_t = tpool.tile([128, 4 * 128], F32, tag="ep_E2")
                    E2 = E2_t[:, : gw * 128]
                    E3_t = tpool.tile([128, 4 * 128], F32, tag="ep_E3")
                    E3 = E3_t[:, : gw * 128]
                    nc.vector.tensor_tensor(
                        out=_r3(E1, 128), in0=_r3(state[:, sl], 128), in1=_bcast_mid(rootr, gw), op=AX.add
                    )
                    nc.scalar.activation(out=E1, in_=E1, func=AF.Relu)
                    nc.vector.tensor_tensor(
                        out=_r3(E2, 128), in0=_r3(E1, 128),
                        in1=_bcast_last(dvc[:, bg : bg + gw], 128), op=AX.mult,
                    )
                    nc.vector.tensor_tensor(
                        out=_r3(E3, 128), in0=_r3(pm[:, : gw * 128], 128),
                        in1=_bcast_last(dic[:, bg : bg + gw], 128), op=AX.mult,
                    )
                    nc.vector.tensor_tensor(out=E2, in0=E3, in1=E2, op=AX.add)
                    nc.vector.tensor_tensor(
                        out=_r3(hdst[:, sl], 128), in0=_r3(E2, 128), in1=_bcast_mid(convbr, gw), op=AX.add
                    )
            # software-pipelined emission: step-1 publishes overlap step-0 consumption
            emit_publish0(0)
            emit_publish0(1)
            emit_groups(0, 0, 6)
            for g in range(0, 6):
                pub_ln_g(g)
            for g in range(0, 6):
                pub_xws_g(1, g)
            pub_ag(1, 0)
            emit_groups(0, 6, NGRP)
            for g in range(6, NGRP):
                pub_ln_g(g)
            for g in range(6, NGRP):
                pub_xws_g(1, g)
            pub_ag(1, 1)
            emit_groups(1, 0, NGRP)

            # ---- output ----
            nc.sync.dma_start(
                out=out_ext[:].rearrange("(w p) f -> p w f", p=128),
                in_=_r3(hA[:], 128),
            )
    nc.compile()
    return nc


def _rep(v):
    return np.tile(np.asarray(v, np.float32).reshape(1, H), (128, 1))


def kernel_with_results(**inputs):
    in_feat = np.asarray(inputs["in_feat"], np.float32)
    row = np.asarray(inputs["row"]).astype(np.int64)
    col = np.asarray(inputs["col"]).astype(np.int64)
    lin_w = np.asarray(inputs["lin_w"], np.float32)
    lin_b = np.asarray(inputs["lin_b"], np.float32)
    conv_w = np.asarray(inputs["conv_w"], np.float32)
    conv_b = np.asarray(inputs["conv_b"], np.float32)
    root_emb = np.asarray(inputs["root_emb"], np.float32)
    ln_gamma = np.asarray(inputs["ln_gamma"], np.float32)
    ln_beta = np.asarray(inputs["ln_beta"], np.float32)

    g = _prep_graph(row, col)
    nc = _build(g["B"], g["gpos"], g["NBLK"])

    ift_full = np.ascontiguousarray(in_feat.T)
    consts = np.concatenate(
        [_rep(lin_b), _rep(root_emb[0]), _rep(conv_b), _rep(ln_gamma[1]), _rep(ln_beta[1])],
        axis=1,
    )
    iota = np.tile(np.arange(128, dtype=np.float32), (128, 1)).astype(BF16)
    lin_wT = np.ascontiguousarray(lin_w.T).astype(BF16)
    conv_wT = np.ascontiguousarray(conv_w.T).astype(BF16)

    in_maps = []
    for k in range(NCORES):
        ift_k = np.zeros((IN, PADN), BF16)
        ift_k[:, :NPC] = ift_full[:, k * NPC : (k + 1) * NPC].astype(BF16)
        m = {
            "ift": ift_k,
            "lin_wT": lin_wT,
            "conv_wT": conv_wT,
            "consts": consts,
            "iota": iota,
            "discols": g["dis_cols"][k],
            "dinvcols": g["dinv_cols"][k],
        }
        for s in range(NSUP):
            m[f"idx{s}"] = g["idx_w"][s][k]
            m[f"tloc{s}"] = np.ascontiguousarray(g["tlocs"][s][k])
        in_maps.append(m)

    res = run_bass_kernel_spmd(nc, in_maps, list(range(NCORES)))
    out = np.concatenate(
        [np.asarray(res.results[k]["out"])[:NPC] for k in range(NCORES)], axis=0
    )
    return out.astype(np.float32), res


def kernel(**inputs):
    out, _ = kernel_with_results(**inputs)
    return out

